# revision 1
# baseline (speedup 1.0000x reference)
"""Graphormer attention head on 8 trn2 NeuronCores (row-parallel).

out = softmax(mask(q@k.T/8, adj)) @ v  with q/k/v = x@W+b, adj scattered
from edge_index.

Sharding: core c owns output rows [c*1024, (c+1)*1024). k/v are computed
replicated on every core from a streamed x^T. The adjacency mask is
precomputed on the host as {0,1} fp8 (transposed, per-core column slice)
and applied multiplicatively AFTER exp: exp(S)*A equals the masked-softmax
numerator exactly (scores are bounded, ~|S|<8, so no row-max subtraction
is needed and exp never overflows; non-edges contribute exactly 0).
The softmax denominator comes free via a ones-column appended to V.
"""
import os
import sys

for _p in ("/opt/trn_rl_repo", "/root/.axon_site/_ro/trn_rl_repo"):
    if os.path.isdir(_p) and _p not in sys.path:
        sys.path.insert(0, _p)

import numpy as np
import ml_dtypes

import concourse.bass as bass
import concourse.bacc as bacc
import concourse.mybir as mybir
import concourse.tile as tile
from concourse.bass_utils import run_bass_kernel_spmd

N = 8192
DIN = 256
DQ = 64
NCORES = 8
NLOC = N // NCORES          # 1024 rows per core
JT = N // 128               # 64 column tiles of 128
SEG = 512                   # fp32 moving-operand max
F32 = mybir.dt.float32
FP8 = mybir.dt.float8e4
AO = None                   # AluOpType, set on import below


def _emit(nc, tc, ctx):
    import contextlib
    from concourse.mybir import AluOpType as AO, ActivationFunctionType as AF

    xt = nc.dram_tensor("xt", [DIN, N], F32, kind="ExternalInput")
    xtq = nc.dram_tensor("xtq", [DIN, NLOC], F32, kind="ExternalInput")
    wq = nc.dram_tensor("wq", [DIN, DQ], F32, kind="ExternalInput")
    wk = nc.dram_tensor("wk", [DIN, DQ], F32, kind="ExternalInput")
    wv = nc.dram_tensor("wv", [DIN, DQ], F32, kind="ExternalInput")
    bq = nc.dram_tensor("bq", [DQ, 1], F32, kind="ExternalInput")
    bk = nc.dram_tensor("bk", [DQ, 1], F32, kind="ExternalInput")
    i65 = nc.dram_tensor("i65", [DQ + 1, DQ + 1], F32, kind="ExternalInput")
    maskt = nc.dram_tensor("maskt", [N, NLOC], FP8, kind="ExternalInput")
    out = nc.dram_tensor("out", [NLOC, DQ], F32, kind="ExternalOutput")

    pers = ctx.enter_context(tc.tile_pool(name="pers", bufs=1))
    pm = ctx.enter_context(tc.tile_pool(name="pm", bufs=4))
    pe_ = ctx.enter_context(tc.tile_pool(name="pe", bufs=3))
    pw = ctx.enter_context(tc.tile_pool(name="pw", bufs=3))
    pfin = ctx.enter_context(tc.tile_pool(name="pfin", bufs=2))
    ps = ctx.enter_context(tc.tile_pool(name="ps", bufs=2, space="PSUM"))
    pacc = ctx.enter_context(tc.tile_pool(name="pacc", bufs=1, space="PSUM"))
    pp = ctx.enter_context(tc.tile_pool(name="pp", bufs=2, space="PSUM"))

    # ---- persistent SBUF ----
    xt_sb = [pers.tile([128, N], F32, tag=f"xt{c}", name=f"xt{c}") for c in range(2)]
    xtq_sb = [pers.tile([128, NLOC], F32, tag=f"xtq{c}", name=f"xtq{c}") for c in range(2)]
    w_sb = {}
    for nm, t in (("wq", wq), ("wk", wk), ("wv", wv)):
        for c in range(2):
            w_sb[nm, c] = pers.tile([128, DQ], F32, tag=f"{nm}{c}", name=f"w{nm}{c}")
            nc.sync.dma_start(w_sb[nm, c][:], t[c * 128:(c + 1) * 128, :])
    bq_sb = pers.tile([DQ, 1], F32, tag="bq")
    bk_sb = pers.tile([DQ, 1], F32, tag="bk")
    i65_sb = pers.tile([DQ + 1, DQ + 1], F32, tag="i65")
    nc.sync.dma_start(bq_sb[:], bq[:])
    nc.sync.dma_start(bk_sb[:], bk[:])
    nc.sync.dma_start(i65_sb[:], i65[:])
    F16 = mybir.dt.float16
    qth_sb = pers.tile([DQ, NLOC], F16, tag="qth")
    qtl_sb = pers.tile([DQ, NLOC], F16, tag="qtl")
    kth_sb = pers.tile([DQ, N], F16, tag="kth")
    ktl_sb = pers.tile([DQ, N], F16, tag="ktl")
    vh_sb = pers.tile([128, JT * (DQ + 1)], F16, tag="vh")
    accT_sb = pers.tile([DQ + 1, NLOC], F32, tag="accT")

    # x^T streamed in 512-col segments so projections can start early
    for c in range(2):
        for s in range(N // SEG):
            nc.sync.dma_start(
                xt_sb[c][:, s * SEG:(s + 1) * SEG],
                xt[c * 128:(c + 1) * 128, s * SEG:(s + 1) * SEG],
            )
        nc.sync.dma_start(xtq_sb[c][:], xtq[c * 128:(c + 1) * 128, :])

    # ---- projections ----
    # Q^T / K^T in fp16 hi+lo pairs (hi = round(q), lo = round(q - hi)) so
    # the scores matmul can run as a 3-term fp16 split (error ~2^-22).
    def _proj_hilo(w_name, xs, ncols, hi, lo, bias):
        for s in range(ncols // SEG):
            t = pp.tile([128, SEG], F32, tag="pp", name=f"pp_{w_name}{s}")
            tp = t[:DQ, :]
            nc.tensor.matmul(tp, w_sb[w_name, 0][:], xs[0][:, s * SEG:(s + 1) * SEG],
                             start=True, stop=False)
            nc.tensor.matmul(tp, w_sb[w_name, 1][:], xs[1][:, s * SEG:(s + 1) * SEG],
                             start=False, stop=True)
            dst = slice(s * SEG, (s + 1) * SEG)
            nc.vector.tensor_scalar_add(hi[:, dst], tp, bias)
            nc.vector.scalar_tensor_tensor(lo[:, dst], tp, bias, hi[:, dst],
                                           AO.add, AO.subtract)

    _proj_hilo("wq", xtq_sb, NLOC, qth_sb, qtl_sb, bq_sb[:])
    _proj_hilo("wk", xt_sb, N, kth_sb, ktl_sb, bk_sb[:])
    # V [8192 x 64] stored j-major as 64 blocks of [128 x 65] (65th col = 1.0
    # for the softmax denominator). bv is folded in at the end via i65.
    vh3 = vh_sb[:].rearrange("p (b e) -> p b e", e=DQ + 1)
    nc.vector.memset(vh3[:, :, DQ:DQ + 1], 1.0)
    for g in range(8):
        t = pp.tile([128, SEG], F32, tag="pp")
        for b in range(8):
            jt = g * 8 + b
            o = t[:, b * DQ:(b + 1) * DQ]
            nc.tensor.matmul(o, xt_sb[0][:, jt * 128:(jt + 1) * 128],
                             w_sb["wv", 0][:], start=True, stop=False)
            nc.tensor.matmul(o, xt_sb[1][:, jt * 128:(jt + 1) * 128],
                             w_sb["wv", 1][:], start=False, stop=True)
        gh = vh3[:, g * 8:(g + 1) * 8, 0:DQ]
        nc.scalar.activation(gh, t[:], AF.Copy)

    # ---- main loop over 64 column tiles ----
    acc = pacc.tile([DQ + 1, NLOC], F32, tag="acc")
    for jt in range(JT):
        m_t = pm.tile([128, NLOC], FP8, tag="m")
        nc.sync.dma_start(m_t[:], maskt[jt * 128:(jt + 1) * 128, :])
        s_t = ps.tile([128, NLOC], F32, tag="s")
        kh = kth_sb[:, jt * 128:(jt + 1) * 128]
        kl = ktl_sb[:, jt * 128:(jt + 1) * 128]
        # 3-term fp16 split; kh stays loaded for 4 matmuls
        for h in range(2):
            hs = slice(h * SEG, (h + 1) * SEG)
            nc.tensor.matmul(s_t[:, hs], kh, qth_sb[:, hs],
                             start=True, stop=False)
            nc.tensor.matmul(s_t[:, hs], kh, qtl_sb[:, hs],
                             start=False, stop=False)
        for h in range(2):
            hs = slice(h * SEG, (h + 1) * SEG)
            nc.tensor.matmul(s_t[:, hs], kl, qth_sb[:, hs],
                             start=False, stop=True)
        e_t = pe_.tile([128, NLOC], F16, tag="e")
        nc.scalar.activation(e_t[:], s_t[:], AF.Exp)
        w_t = pw.tile([128, NLOC], F16, tag="w")
        nc.vector.scalar_tensor_tensor(w_t[:], e_t[:], 1.0, m_t[:],
                                       AO.mult, AO.mult)
        vhb = vh3[:, jt, :]
        for h in range(2):
            hs = slice(h * SEG, (h + 1) * SEG)
            nc.tensor.matmul(acc[:, hs], vhb, w_t[:, hs],
                             start=(jt == 0), stop=(jt == JT - 1))

    # ---- finish: transpose via matmul with I65 (adds bv*Z), divide by Z ----
    nc.scalar.activation(accT_sb[:], acc[:], AF.Copy)
    for it in range(NLOC // 128):
        po = pp.tile([128, DQ + 1], F32, tag="pp")
        nc.tensor.matmul(po[:], accT_sb[:, it * 128:(it + 1) * 128], i65_sb[:],
                         start=True, stop=True)
        rz = pfin.tile([128, 1], F32, tag="rz")
        nc.vector.reciprocal(rz[:], po[:, DQ:DQ + 1])
        o_t = pfin.tile([128, DQ], F32, tag="o")
        nc.vector.tensor_scalar_mul(o_t[:], po[:, 0:DQ], rz[:])
        nc.sync.dma_start(out[it * 128:(it + 1) * 128, :], o_t[:])


_CACHE = {}


def _program():
    if "nc" not in _CACHE:
        import contextlib
        nc = bacc.Bacc("TRN2", target_bir_lowering=False, debug=False,
                       num_devices=NCORES)
        with tile.TileContext(nc) as tc:
            with contextlib.ExitStack() as ctx:
                _emit(nc, tc, ctx)
        nc.compile()
        _CACHE["nc"] = nc
    return _CACHE["nc"]


def kernel(**inputs):
    x = np.asarray(inputs["x"], dtype=np.float32)
    ei = np.asarray(inputs["edge_index"])
    Wq = np.asarray(inputs["Wq"], dtype=np.float32)
    bq = np.asarray(inputs["bq"], dtype=np.float32)
    Wk = np.asarray(inputs["Wk"], dtype=np.float32)
    bk = np.asarray(inputs["bk"], dtype=np.float32)
    Wv = np.asarray(inputs["Wv"], dtype=np.float32)
    bv = np.asarray(inputs["bv"], dtype=np.float32)

    scale = 1.0 / np.sqrt(np.float32(DQ))
    xT = np.ascontiguousarray(x.T)                      # (256, 8192)
    wq_s = np.ascontiguousarray(Wq * scale)
    bq_s = np.ascontiguousarray((bq * scale).reshape(DQ, 1))
    bk_c = np.ascontiguousarray(bk.reshape(DQ, 1))
    i65 = np.eye(DQ + 1, dtype=np.float32)
    i65[DQ, :DQ] = bv
    adj = np.zeros((N, N), dtype=np.bool_)
    adj[ei[0], ei[1]] = True

    in_maps = []
    for c in range(NCORES):
        rows = slice(c * NLOC, (c + 1) * NLOC)
        in_maps.append({
            "xt": xT,
            "xtq": np.ascontiguousarray(xT[:, rows]),
            "wq": wq_s, "wk": Wk, "wv": Wv,
            "bq": bq_s, "bk": bk_c, "i65": i65,
            "maskt": np.ascontiguousarray(adj[rows].T).astype(
                ml_dtypes.float8_e4m3),
        })

    global _last_in_maps
    _last_in_maps = in_maps
    nc = _program()
    res = run_bass_kernel_spmd(nc, in_maps, core_ids=list(range(NCORES)))
    out = np.concatenate([res.results[c]["out"] for c in range(NCORES)], axis=0)
    return out.astype(np.float32)


_last_in_maps = None



# revision 6
# speedup vs baseline: 1.6986x; 1.6986x over previous
"""Graphormer attention head on 8 trn2 NeuronCores (row-parallel).

out = softmax(mask(q@k.T/8, adj)) @ v  with q/k/v = x@W+b, adj scattered
from edge_index.  Core c owns output rows [c*1024, (c+1)*1024).

Design (per core, all under the Act-engine exp shadow ~66us):
- Projections in fp16 (4x faster than fp32 matmul). No bias adds on
  device: per-q-row bias terms cancel in softmax; the per-k-row term
  c_j = (bq*scale)@k_j is computed as an extra output column of the V
  projection and applied as the per-partition bias of the Exp
  activation.
- Scores: q,k split hi+lo in fp8e4 (host keeps fp16-grade accuracy);
  one DoubleRow matmul per [128,512] output computes the full
  (qh+ql)(kh+kl) product: contraction packs dims 0-63 as partitions
  0-63 with (hi,hi)/(lo,lo) pairing and duplicates them at partitions
  64-127 with the q pairing swapped.
- Mask: additive offsets {-2 edge, -28 non-edge} in fp8, added into
  the scores PSUM by DoubleRow matmuls with (I,0)/(0,I) identity
  stationaries (a mask pair tile carries two adjacent jt tiles).
  The -2 global shift keeps exp outputs within fp8e4 range.
- Exp on Act engine reads PSUM f32, writes w directly as fp8e4 into
  the paired layout consumed by attn@v. No per-tile DVE work at all.
- attn@v: two DoubleRow passes (v-hi, v-lo in fp8e4) contract 256
  k-rows per instruction; a ones column in V yields the softmax
  denominator, a final small matmul with I66 transposes and folds bv.
"""
import os
import sys

for _p in ("/opt/trn_rl_repo", "/root/.axon_site/_ro/trn_rl_repo"):
    if os.path.isdir(_p) and _p not in sys.path:
        sys.path.insert(0, _p)

import numpy as np
import ml_dtypes

import concourse.bass as bass
import concourse.bacc as bacc
import concourse.mybir as mybir
import concourse.tile as tile
from concourse.bass_utils import run_bass_kernel_spmd

N = 8192
DIN = 256
DQ = 64
NCORES = 8
NLOC = N // NCORES          # 1024 rows per core
JT = N // 128               # 64 column tiles of 128
NPAIR = JT // 2             # 32 pairs of tiles
M = DQ + 2                  # v cols + ones col (64) + c col (65)
MB = 80                     # padded v-block stride (DoubleRow needs step%16==0)
SHIFT = -2.0                # global score shift (cancels in softmax)
MOFF = -28.0                # additive mask for non-edges
F32 = mybir.dt.float32
F16 = mybir.dt.float16
FP8 = mybir.dt.float8e4


def _emit(nc, tc, ctx):
    from concourse.mybir import AluOpType as AO, ActivationFunctionType as AF
    DR = mybir.MatmulPerfMode.DoubleRow

    xt = nc.dram_tensor("xt", [DIN, N], F16, kind="ExternalInput")
    xtq = nc.dram_tensor("xtq", [DIN, NLOC], F16, kind="ExternalInput")
    wq = nc.dram_tensor("wq", [DIN, DQ], F16, kind="ExternalInput")
    wk = nc.dram_tensor("wk", [DIN, DQ], F16, kind="ExternalInput")
    wv = nc.dram_tensor("wv", [DIN, M], F16, kind="ExternalInput")
    mask3 = nc.dram_tensor("mask3", [128, JT * NLOC], FP8, kind="ExternalInput")
    id2 = nc.dram_tensor("id2", [128, 512], FP8, kind="ExternalInput")
    i66 = nc.dram_tensor("i66", [M, M], F32, kind="ExternalInput")
    out = nc.dram_tensor("out", [NLOC, DQ], F32, kind="ExternalOutput")

    pers = ctx.enter_context(tc.tile_pool(name="pers", bufs=1))
    pm = ctx.enter_context(tc.tile_pool(name="pm", bufs=4))
    pw = ctx.enter_context(tc.tile_pool(name="pw", bufs=3))
    pfin = ctx.enter_context(tc.tile_pool(name="pfin", bufs=2))
    ps = ctx.enter_context(tc.tile_pool(name="ps", bufs=2, space="PSUM"))
    pacc = ctx.enter_context(tc.tile_pool(name="pacc", bufs=1, space="PSUM"))
    pp = ctx.enter_context(tc.tile_pool(name="pp", bufs=2, space="PSUM"))

    # ---- persistent SBUF ----
    xt_sb = [pers.tile([128, N], F16, tag=f"xt{c}", name=f"xt{c}") for c in range(2)]
    xtq_sb = [pers.tile([128, NLOC], F16, tag=f"xtq{c}", name=f"xtq{c}") for c in range(2)]
    w_sb = {}
    for nm, t, wid in (("wq", wq, DQ), ("wk", wk, DQ), ("wv", wv, M)):
        for c in range(2):
            w_sb[nm, c] = pers.tile([128, wid], F16, tag=f"{nm}{c}", name=f"w{nm}{c}")
            nc.sync.dma_start(w_sb[nm, c][:], t[c * 128:(c + 1) * 128, :])
    id2_sb = pers.tile([128, 512], FP8, tag="id2")
    nc.sync.dma_start(id2_sb[:], id2[:])
    idv = id2_sb[:].rearrange("p (a i m) -> p a i m", a=2, i=2)
    i66_sb = pers.tile([M, M], F32, tag="i66")
    nc.sync.dma_start(i66_sb[:], i66[:])

    kk_sb = pers.tile([128, 2, N], FP8, tag="kk")       # (hi,lo) pairs, dup'd
    q3_sb = pers.tile([128, 2, NLOC], FP8, tag="q3")    # (hi,lo)/(lo,hi)
    vh_sb = pers.tile([128, JT * M], F16, tag="vh")     # v blocks [128,66]
    c_sb = pers.tile([128, JT], F32, tag="c")
    accT_sb = pers.tile([M, NLOC], F32, tag="accT")
    wu_sb = pers.tile([128, 8], F16, tag="wu")

    vh3 = vh_sb[:].rearrange("p (b e) -> p b e", e=M)

    # warm the Exp activation table before the main loop needs it
    nc.vector.memset(wu_sb[:], 0.0)
    nc.scalar.activation(wu_sb[:], wu_sb[:], AF.Exp)

    # ---- Q projection -> q3 (fp8 hi/lo, swapped duplicate in p64-127) ----
    for c in range(2):
        nc.sync.dma_start(xtq_sb[c][:], xtq[c * 128:(c + 1) * 128, :])
    for h in range(2):
        hs = slice(h * 512, (h + 1) * 512)
        t = pp.tile([128, 512], F32, tag="pp", name=f"q{h}")
        tp = t[:DQ, :]
        nc.tensor.matmul(tp, w_sb["wq", 0][:], xtq_sb[0][:, hs],
                         start=True, stop=False)
        nc.tensor.matmul(tp, w_sb["wq", 1][:], xtq_sb[1][:, hs],
                         start=False, stop=True)
        nc.vector.tensor_scalar_add(q3_sb[0:64, 0, hs], tp, 0.0)
        nc.vector.scalar_tensor_tensor(q3_sb[0:64, 1, hs], tp, 1.0,
                                       q3_sb[0:64, 0, hs],
                                       AO.mult, AO.subtract)
    nc.sync.dma_start(q3_sb[64:128, 0, :], q3_sb[0:64, 1, :])
    nc.sync.dma_start(q3_sb[64:128, 1, :], q3_sb[0:64, 0, :])

    # ---- K and V projections, streamed per 1024-col segment of x^T ----
    for s in range(8):
        sseg = slice(s * 1024, (s + 1) * 1024)
        for c in range(2):
            nc.sync.dma_start(xt_sb[c][:, sseg],
                              xt[c * 128:(c + 1) * 128, sseg])
        # K: two 512 halves -> kk (hi,lo) at partitions 0-63
        for hh in range(2):
            cols = slice(s * 1024 + hh * 512, s * 1024 + (hh + 1) * 512)
            t = pp.tile([128, 512], F32, tag="pp", name=f"k{s}_{hh}")
            tp = t[:DQ, :]
            nc.tensor.matmul(tp, w_sb["wk", 0][:], xt_sb[0][:, cols],
                             start=True, stop=False)
            nc.tensor.matmul(tp, w_sb["wk", 1][:], xt_sb[1][:, cols],
                             start=False, stop=True)
            nc.vector.tensor_scalar_add(kk_sb[0:64, 0, cols], tp, 0.0)
            nc.vector.scalar_tensor_tensor(kk_sb[0:64, 1, cols], tp, 1.0,
                                           kk_sb[0:64, 0, cols],
                                           AO.mult, AO.subtract)
        # V: blocks 8s..8s+7 in two groups of 4; 65th col = c_j source
        for g2 in range(2):
            b0 = 8 * s + 4 * g2
            t = pp.tile([128, 4 * M], F32, tag="pp", name=f"v{s}_{g2}")
            for b in range(4):
                blk = b0 + b
                o = t[:, b * M:(b + 1) * M]
                nc.tensor.matmul(o, xt_sb[0][:, blk * 128:(blk + 1) * 128],
                                 w_sb["wv", 0][:], start=True, stop=False)
                nc.tensor.matmul(o, xt_sb[1][:, blk * 128:(blk + 1) * 128],
                                 w_sb["wv", 1][:], start=False, stop=True)
            t3 = t[:].rearrange("p (b e) -> p b e", e=M)
            nc.vector.tensor_scalar_add(vh_sb[:, b0 * M:(b0 + 4) * M], t[:],
                                        0.0)
            nc.vector.tensor_scalar_add(c_sb[:, b0:b0 + 4], t3[:, :, M - 1],
                                        0.0)
            nc.vector.memset(vh3[:, b0:b0 + 4, DQ:DQ + 1], 1.0)
        # duplicate kk (hi,lo) into partitions 64-127 (same order; the
        # swapped pairing lives on the q side)
        for i in range(2):
            nc.sync.dma_start(kk_sb[64:128, i, sseg], kk_sb[0:64, i, sseg])

    # ---- main loop over 64 k-tiles of 128 rows ----
    acc = pacc.tile([M, NLOC], F32, tag="acc")
    m_t = None
    wts = [None] * JT

    def _emit_av(j):
        for h in range(2):
            hs = slice(h * 512, (h + 1) * 512)
            nc.tensor.matmul(acc[:, hs], vh3[:, j], wts[j][:, hs],
                             start=(j == 0), stop=(j == JT - 1))

    for jt in range(JT):
        pr = jt // 2
        if jt % 2 == 0:
            m_t = pm.tile([128, 2, NLOC], FP8, tag="m", name=f"m{pr}")
            nc.sync.dma_start(m_t[:].rearrange("p i n -> p (i n)"),
                              mask3[:, pr * 2048:(pr + 1) * 2048])
        s_t = ps.tile([128, NLOC], F32, tag="s", name=f"s{jt}")
        kts = kk_sb[:, :, jt * 128:(jt + 1) * 128]
        for h in range(2):
            hs = slice(h * 512, (h + 1) * 512)
            nc.tensor.matmul(s_t[:, hs], kts, q3_sb[:, :, hs],
                             start=True, stop=False, perf_mode=DR)
        for h in range(2):
            hs = slice(h * 512, (h + 1) * 512)
            nc.tensor.matmul(s_t[:, hs], idv[:, jt % 2], m_t[:, :, hs],
                             start=False, stop=True, perf_mode=DR)
        w_t = pw.tile([128, NLOC], F16, tag="w", name=f"w{jt}")
        wts[jt] = w_t
        nc.scalar.activation(w_t[:], s_t[:], AF.Exp,
                             bias=c_sb[:, jt:jt + 1])
        if jt >= 1:
            _emit_av(jt - 1)
    _emit_av(JT - 1)

    # ---- finish: transpose via matmul with I66 (adds bv*Z), divide ----
    nc.vector.tensor_scalar_add(accT_sb[:], acc[:], 0.0)
    for it in range(NLOC // 128):
        po = pp.tile([128, M], F32, tag="pp", name=f"po{it}")
        nc.tensor.matmul(po[:], accT_sb[:, it * 128:(it + 1) * 128],
                         i66_sb[:], start=True, stop=True)
        rz = pfin.tile([128, 1], F32, tag="rz")
        nc.vector.reciprocal(rz[:], po[:, DQ:DQ + 1])
        o_t = pfin.tile([128, DQ], F32, tag="o")
        nc.vector.tensor_scalar_mul(o_t[:], po[:, 0:DQ], rz[:])
        nc.sync.dma_start(out[it * 128:(it + 1) * 128, :], o_t[:])


_CACHE = {}


def _program():
    if "nc" not in _CACHE:
        import contextlib
        nc = bacc.Bacc("TRN2", target_bir_lowering=False, debug=False,
                       num_devices=NCORES)
        with tile.TileContext(nc) as tc:
            with contextlib.ExitStack() as ctx:
                _emit(nc, tc, ctx)
        nc.compile()
        _CACHE["nc"] = nc
    return _CACHE["nc"]


def kernel(**inputs):
    x = np.asarray(inputs["x"], dtype=np.float32)
    ei = np.asarray(inputs["edge_index"])
    Wq = np.asarray(inputs["Wq"], dtype=np.float32)
    bq = np.asarray(inputs["bq"], dtype=np.float32)
    Wk = np.asarray(inputs["Wk"], dtype=np.float32)
    bk = np.asarray(inputs["bk"], dtype=np.float32)
    Wv = np.asarray(inputs["Wv"], dtype=np.float32)
    bv = np.asarray(inputs["bv"], dtype=np.float32)
    del bk  # its score contribution is constant per q row: cancels

    FP8NP = ml_dtypes.float8_e4m3
    scale = np.float32(1.0 / np.sqrt(np.float32(DQ)))
    xT16 = np.ascontiguousarray(x.T.astype(np.float16))
    wq_s = np.ascontiguousarray((Wq * scale).astype(np.float16))
    wk16 = np.ascontiguousarray(Wk.astype(np.float16))
    w_c = Wk @ (bq * scale)                      # c_j = (bq*s) . k_j
    wv_aug = np.zeros((DIN, M), np.float32)
    wv_aug[:, :DQ] = Wv
    wv_aug[:, DQ + 1] = w_c
    wv16 = np.ascontiguousarray(wv_aug.astype(np.float16))
    i66 = np.zeros((M, M), np.float32)
    i66[np.arange(DQ), np.arange(DQ)] = 1.0
    i66[DQ, :DQ] = bv
    i66[DQ, DQ] = 1.0
    id2 = np.zeros((128, 2, 2, 128), np.float32)
    r = np.arange(128)
    id2[r, 0, 0, r] = 1.0
    id2[r, 1, 1, r] = 1.0
    id2 = np.ascontiguousarray(id2.astype(FP8NP).reshape(128, 512))

    adj = np.zeros((N, N), dtype=np.bool_)
    adj[ei[0], ei[1]] = True

    in_maps = []
    for c in range(NCORES):
        rows = slice(c * NLOC, (c + 1) * NLOC)
        moff = np.where(adj[rows].T, np.float32(SHIFT), np.float32(MOFF))
        m3 = np.ascontiguousarray(
            moff.reshape(NPAIR, 2, 128, NLOC).transpose(2, 0, 1, 3)
            .astype(FP8NP).reshape(128, JT * NLOC))
        in_maps.append({
            "xt": xT16,
            "xtq": np.ascontiguousarray(xT16[:, rows]),
            "wq": wq_s, "wk": wk16, "wv": wv16,
            "mask3": m3, "id2": id2, "i66": i66,
        })

    global _last_in_maps
    _last_in_maps = in_maps
    nc = _program()
    res = run_bass_kernel_spmd(nc, in_maps, core_ids=list(range(NCORES)))
    out = np.concatenate([res.results[c]["out"] for c in range(NCORES)], axis=0)
    return out.astype(np.float32)


_last_in_maps = None


# revision 9
# speedup vs baseline: 1.7913x; 1.0546x over previous
"""Graphormer attention head on 8 trn2 NeuronCores (row-parallel).

out = softmax(mask(q@k.T/8, adj)) @ v  with q/k/v = x@W+b, adj scattered
from edge_index.  Core c owns output rows [c*1024, (c+1)*1024).

Design (per core):
- Projections in fp16. No bias adds on device: per-q-row bias terms
  cancel in softmax; the per-k-row term c_j = (bq*scale)@k_j is an
  extra output column of the V projection, applied as the per-partition
  bias of the Exp activation.
- Scores: q,k split hi+lo in fp8e4; one DoubleRow matmul per [128,512]
  output computes the full (qh+ql)(kh+kl) product: dims 0-63 sit at
  partitions 0-63 with (hi,hi)/(lo,lo) pairing and are duplicated at
  partitions 64-127 with the q pairing swapped.  The K/Q projections
  use column-duplicated weights so the PSUM result is already
  replicated across both partition halves.
- Mask: additive offsets {-2 edge, -28 non-edge} in fp8, added into
  the scores PSUM by DoubleRow matmuls with (I,0)/(0,I) identity
  stationaries (a mask tile carries two adjacent jt tiles). The -2
  shift is a softmax-invariant that bounds exp outputs.
- Exp on Act engine (the ~66us floor): PSUM f32 -> fp16 w tiles,
  with the c_j bias fused.  No per-tile DVE work.
- attn@v in fp16: one matmul per tile accumulating [66,1024]; a ones
  column in V yields the softmax denominator; a final small matmul
  with I66 transposes and folds bv.
- DMA triggers cost ~600ns of queue time each, so transfers are
  batched (one per x^T segment, one per 4 mask tiles) and mask loads
  issue from the otherwise-idle GpSimd queue; projection work for
  x^T segment s+1 is emitted inside main-loop block s-1 so the PE
  stream never stalls the Act engine.
"""
import os
import sys

for _p in ("/opt/trn_rl_repo", "/root/.axon_site/_ro/trn_rl_repo"):
    if os.path.isdir(_p) and _p not in sys.path:
        sys.path.insert(0, _p)

import numpy as np
import ml_dtypes

import concourse.bass as bass
import concourse.bacc as bacc
import concourse.mybir as mybir
import concourse.tile as tile
from concourse.bass_utils import run_bass_kernel_spmd

N = 8192
DIN = 256
DQ = 64
NCORES = 8
NLOC = N // NCORES          # 1024 rows per core
JT = N // 128               # 64 column tiles of 128
NSEG = 8                    # x^T streamed in 8 segments of 1024 columns
M = DQ + 2                  # v cols + ones col (64) + c col (65)
SHIFT = -2.0                # global score shift (cancels in softmax)
MOFF = -28.0                # additive mask for non-edges
F32 = mybir.dt.float32
F16 = mybir.dt.float16
FP8 = mybir.dt.float8e4


def _emit(nc, tc, ctx):
    from concourse.mybir import AluOpType as AO, ActivationFunctionType as AF
    DR = mybir.MatmulPerfMode.DoubleRow

    xt3 = nc.dram_tensor("xt3", [128, NSEG * 2 * 1024], F16,
                         kind="ExternalInput")
    xtq3 = nc.dram_tensor("xtq3", [128, 2 * NLOC], F16, kind="ExternalInput")
    wq3 = nc.dram_tensor("wq3", [128, 2 * 128], F16, kind="ExternalInput")
    wk3 = nc.dram_tensor("wk3", [128, 2 * 128], F16, kind="ExternalInput")
    wv3 = nc.dram_tensor("wv3", [128, 2 * M], F16, kind="ExternalInput")
    mask3 = nc.dram_tensor("mask3", [128, JT * NLOC], FP8,
                           kind="ExternalInput")
    id2 = nc.dram_tensor("id2", [128, 512], FP8, kind="ExternalInput")
    i66 = nc.dram_tensor("i66", [M, M], F32, kind="ExternalInput")
    out = nc.dram_tensor("out", [NLOC, DQ], F32, kind="ExternalOutput")

    pers = ctx.enter_context(tc.tile_pool(name="pers", bufs=1))
    pm = ctx.enter_context(tc.tile_pool(name="pm", bufs=3))
    pw = ctx.enter_context(tc.tile_pool(name="pw", bufs=3))
    pfin = ctx.enter_context(tc.tile_pool(name="pfin", bufs=2))
    ps = ctx.enter_context(tc.tile_pool(name="ps", bufs=2, space="PSUM"))
    pacc = ctx.enter_context(tc.tile_pool(name="pacc", bufs=1, space="PSUM"))
    pp = ctx.enter_context(tc.tile_pool(name="pp", bufs=2, space="PSUM"))

    # ---- persistent SBUF ----
    xt_sb = pers.tile([128, NSEG, 2, 1024], F16, tag="xt")
    xtq_sb = pers.tile([128, 2, NLOC], F16, tag="xtq")
    wq_sb = pers.tile([128, 2, 128], F16, tag="wq")
    wk_sb = pers.tile([128, 2, 128], F16, tag="wk")
    wv_sb = pers.tile([128, 2, M], F16, tag="wv")
    id2_sb = pers.tile([128, 512], FP8, tag="id2")
    i66_sb = pers.tile([M, M], F32, tag="i66")
    kk_sb = pers.tile([128, 2, N], FP8, tag="kk")      # (hi,lo), both halves
    q3_sb = pers.tile([128, 2, NLOC], FP8, tag="q3")   # (hi,lo)/(lo,hi)
    vh_sb = pers.tile([128, JT * M], F16, tag="vh")    # v blocks [128,66]
    c_sb = pers.tile([128, JT], F32, tag="c")
    accT_sb = pers.tile([M, NLOC], F32, tag="accT")
    o_all = pers.tile([128, (NLOC // 128) * DQ], F32, tag="oall")
    wu_sb = pers.tile([128, 8], F16, tag="wu")

    idv = id2_sb[:].rearrange("p (a i m) -> p a i m", a=2, i=2)
    vh3 = vh_sb[:].rearrange("p (b e) -> p b e", e=M)

    nc.sync.dma_start(wq_sb[:].rearrange("p c j -> p (c j)"), wq3[:])
    nc.sync.dma_start(wk_sb[:].rearrange("p c j -> p (c j)"), wk3[:])
    nc.sync.dma_start(wv_sb[:].rearrange("p c j -> p (c j)"), wv3[:])
    nc.sync.dma_start(id2_sb[:], id2[:])
    nc.sync.dma_start(i66_sb[:], i66[:])
    nc.sync.dma_start(xtq_sb[:].rearrange("p c n -> p (c n)"), xtq3[:])

    # warm the Exp table before the main loop needs it
    nc.vector.memset(wu_sb[:], 0.0)
    nc.scalar.activation(wu_sb[:], wu_sb[:], AF.Exp)

    # ---- Q projection -> q3 (weights column-duplicated: PSUM rows
    # 64-127 replicate rows 0-63, so the swapped fp8 pairing needs no
    # partition-crossing copies) ----
    for h in range(2):
        hs = slice(h * 512, (h + 1) * 512)
        t = pp.tile([128, 512], F32, tag="pp", name=f"q{h}")
        nc.tensor.matmul(t[:], wq_sb[:, 0, :], xtq_sb[:, 0, hs],
                         start=True, stop=False)
        nc.tensor.matmul(t[:], wq_sb[:, 1, :], xtq_sb[:, 1, hs],
                         start=False, stop=True)
        nc.vector.tensor_scalar_add(q3_sb[0:64, 0, hs], t[0:64, :], 0.0)
        nc.vector.scalar_tensor_tensor(q3_sb[0:64, 1, hs], t[0:64, :], 1.0,
                                       q3_sb[0:64, 0, hs],
                                       AO.mult, AO.subtract)
        nc.vector.tensor_scalar_add(q3_sb[64:128, 1, hs], t[64:128, :], 0.0)
        nc.vector.scalar_tensor_tensor(q3_sb[64:128, 0, hs], t[64:128, :],
                                       1.0, q3_sb[64:128, 1, hs],
                                       AO.mult, AO.subtract)

    def prep_seg(s):
        nc.sync.dma_start(xt_sb[:, s, :, :].rearrange("p c n -> p (c n)"),
                          xt3[:, s * 2048:(s + 1) * 2048])
        # K: two 512-col halves; wk column-duplicated -> [128,512] PSUM
        for hh in range(2):
            cols = slice(s * 1024 + hh * 512, s * 1024 + (hh + 1) * 512)
            xs = slice(hh * 512, (hh + 1) * 512)
            t = pp.tile([128, 512], F32, tag="pp", name=f"k{s}_{hh}")
            nc.tensor.matmul(t[:], wk_sb[:, 0, :], xt_sb[:, s, 0, xs],
                             start=True, stop=False)
            nc.tensor.matmul(t[:], wk_sb[:, 1, :], xt_sb[:, s, 1, xs],
                             start=False, stop=True)
            nc.vector.tensor_scalar_add(kk_sb[:, 0, cols], t[:], 0.0)
            nc.vector.scalar_tensor_tensor(kk_sb[:, 1, cols], t[:], 1.0,
                                           kk_sb[:, 0, cols],
                                           AO.mult, AO.subtract)
        # V: blocks 8s..8s+7 in two groups of 4 (+ c column extraction)
        for g2 in range(2):
            b0 = 8 * s + 4 * g2
            t = pp.tile([128, 4 * M], F32, tag="pp", name=f"v{s}_{g2}")
            for b in range(4):
                xs = slice((4 * g2 + b) * 128, (4 * g2 + b + 1) * 128)
                o = t[:, b * M:(b + 1) * M]
                nc.tensor.matmul(o, xt_sb[:, s, 0, xs], wv_sb[:, 0, :],
                                 start=True, stop=False)
                nc.tensor.matmul(o, xt_sb[:, s, 1, xs], wv_sb[:, 1, :],
                                 start=False, stop=True)
            t3 = t[:].rearrange("p (b e) -> p b e", e=M)
            nc.vector.tensor_scalar_add(vh_sb[:, b0 * M:(b0 + 4) * M], t[:],
                                        0.0)
            nc.vector.tensor_scalar_add(c_sb[:, b0:b0 + 4], t3[:, :, M - 1],
                                        0.0)
            nc.vector.memset(vh3[:, b0:b0 + 4, DQ:DQ + 1], 1.0)

    def mask_dma(q4):
        m_t = pm.tile([128, 4096], FP8, tag="m", name=f"m{q4}")
        nc.gpsimd.dma_start(m_t[:], mask3[:, q4 * 4096:(q4 + 1) * 4096])
        return m_t[:].rearrange("p (t i n) -> p t i n", t=2, i=2)

    mview = [None] * (JT // 4)
    mview[0] = mask_dma(0)
    mview[1] = mask_dma(1)
    prep_seg(0)
    prep_seg(1)

    # ---- main loop over 64 k-tiles of 128 rows ----
    acc = pacc.tile([M, NLOC], F32, tag="acc")
    wts = [None] * JT

    def _emit_av(j):
        for h in range(2):
            hs = slice(h * 512, (h + 1) * 512)
            nc.tensor.matmul(acc[:, hs], vh3[:, j], wts[j][:, hs],
                             start=(j == 0), stop=(j == JT - 1))

    for jt in range(JT):
        if jt % 8 == 0 and 8 <= jt:
            b = jt // 8
            if b + 1 < NSEG:
                prep_seg(b + 1)
        if jt % 4 == 0 and jt // 4 + 2 < JT // 4:
            mview[jt // 4 + 2] = mask_dma(jt // 4 + 2)
        mv = mview[jt // 4]
        s_t = ps.tile([128, NLOC], F32, tag="s", name=f"s{jt}")
        kts = kk_sb[:, :, jt * 128:(jt + 1) * 128]
        for h in range(2):
            hs = slice(h * 512, (h + 1) * 512)
            nc.tensor.matmul(s_t[:, hs], kts, q3_sb[:, :, hs],
                             start=True, stop=False, perf_mode=DR)
        for h in range(2):
            hs = slice(h * 512, (h + 1) * 512)
            nc.tensor.matmul(s_t[:, hs], idv[:, jt % 2],
                             mv[:, (jt % 4) // 2, :, hs],
                             start=False, stop=True, perf_mode=DR)
        w_t = pw.tile([128, NLOC], F16, tag="w", name=f"w{jt}")
        wts[jt] = w_t
        nc.scalar.activation(w_t[:], s_t[:], AF.Exp, bias=c_sb[:, jt:jt + 1])
        if jt >= 1:
            _emit_av(jt - 1)
    _emit_av(JT - 1)

    # ---- finish: transpose via matmul with I66 (adds bv*Z), divide ----
    nc.vector.tensor_scalar_add(accT_sb[:], acc[:], 0.0)
    for it in range(NLOC // 128):
        po = pp.tile([128, M], F32, tag="pp", name=f"po{it}")
        nc.tensor.matmul(po[:], accT_sb[:, it * 128:(it + 1) * 128],
                         i66_sb[:], start=True, stop=True)
        rz = pfin.tile([128, 1], F32, tag="rz")
        nc.vector.reciprocal(rz[:], po[:, DQ:DQ + 1])
        nc.vector.tensor_scalar_mul(o_all[:, it * DQ:(it + 1) * DQ],
                                    po[:, 0:DQ], rz[:])
    nc.sync.dma_start(out[:].rearrange("(i p) c -> p i c", p=128),
                      o_all[:].rearrange("p (i c) -> p i c", c=DQ))


_CACHE = {}


def _program():
    if "nc" not in _CACHE:
        import contextlib
        nc = bacc.Bacc("TRN2", target_bir_lowering=False, debug=False,
                       num_devices=NCORES)
        with tile.TileContext(nc) as tc:
            with contextlib.ExitStack() as ctx:
                _emit(nc, tc, ctx)
        nc.compile()
        _CACHE["nc"] = nc
    return _CACHE["nc"]


def _chunk2(w):
    """[256, width] -> [128, 2*width] with w3[p, c*width+j] = w[c*128+p, j]."""
    width = w.shape[1]
    return np.ascontiguousarray(
        w.reshape(2, 128, width).transpose(1, 0, 2).reshape(128, 2 * width))


def kernel(**inputs):
    x = np.asarray(inputs["x"], dtype=np.float32)
    ei = np.asarray(inputs["edge_index"])
    Wq = np.asarray(inputs["Wq"], dtype=np.float32)
    bq = np.asarray(inputs["bq"], dtype=np.float32)
    Wk = np.asarray(inputs["Wk"], dtype=np.float32)
    Wv = np.asarray(inputs["Wv"], dtype=np.float32)
    bv = np.asarray(inputs["bv"], dtype=np.float32)
    # bk's score contribution is constant per q row: cancels in softmax

    FP8NP = ml_dtypes.float8_e4m3
    scale = np.float32(1.0 / np.sqrt(np.float32(DQ)))
    xT16 = x.T.astype(np.float16)                       # (256, 8192)
    wq_s = (Wq * scale).astype(np.float16)
    wk16 = Wk.astype(np.float16)
    wq3 = _chunk2(np.concatenate([wq_s, wq_s], axis=1))
    wk3 = _chunk2(np.concatenate([wk16, wk16], axis=1))
    w_c = Wk @ (bq * scale)                             # c_j = (bq*s).k_j
    wv_aug = np.zeros((DIN, M), np.float32)
    wv_aug[:, :DQ] = Wv
    wv_aug[:, DQ + 1] = w_c
    wv3 = _chunk2(wv_aug.astype(np.float16))
    xt3 = np.ascontiguousarray(
        xT16.reshape(2, 128, NSEG, 1024).transpose(1, 2, 0, 3)
        .reshape(128, NSEG * 2 * 1024))
    i66 = np.zeros((M, M), np.float32)
    i66[np.arange(DQ), np.arange(DQ)] = 1.0
    i66[DQ, :DQ] = bv
    i66[DQ, DQ] = 1.0
    id2 = np.zeros((128, 2, 2, 128), np.float32)
    r = np.arange(128)
    id2[r, 0, 0, r] = 1.0
    id2[r, 1, 1, r] = 1.0
    id2 = np.ascontiguousarray(id2.astype(FP8NP).reshape(128, 512))

    adj = np.zeros((N, N), dtype=np.bool_)
    adj[ei[0], ei[1]] = True

    in_maps = []
    for c in range(NCORES):
        rows = slice(c * NLOC, (c + 1) * NLOC)
        moff = np.where(adj[rows].T, np.float32(SHIFT), np.float32(MOFF))
        m3 = np.ascontiguousarray(
            moff.reshape(JT // 2, 2, 128, NLOC).transpose(2, 0, 1, 3)
            .astype(FP8NP).reshape(128, JT * NLOC))
        in_maps.append({
            "xt3": xt3,
            "xtq3": _chunk2(np.ascontiguousarray(xT16[:, rows])),
            "wq3": wq3, "wk3": wk3, "wv3": wv3,
            "mask3": m3, "id2": id2, "i66": i66,
        })

    global _last_in_maps
    _last_in_maps = in_maps
    nc = _program()
    res = run_bass_kernel_spmd(nc, in_maps, core_ids=list(range(NCORES)))
    out = np.concatenate([res.results[c]["out"] for c in range(NCORES)],
                         axis=0)
    return out.astype(np.float32)


_last_in_maps = None
